# revision 42
# baseline (speedup 1.0000x reference)
"""Trainium2 Bass kernel for capsule-network dynamic routing.

Problem: u [64, 2048, 16], W [2048, 16, 1024] ->
  uhat = einsum('bni,nij->bnj', u, W)  (viewed [B, N, 32, 32])
  3 routing iterations (softmax over out-caps, squash) -> v [64, 32, 32]

Sharding: n (input capsules) split across 8 cores, 256 per core.
W slice stays SBUF-resident (bf16); uhat is recomputed on the PE each
routing pass (never materialized to HBM).  The per-iteration s-reduction
([64, 1024] partial sums) is AllReduced across cores.

Layout: j is stored k-major (j' = k*32 + o, "(k,o)") so the c-weighting
(t2 = uh * c) broadcasts c over k with a packed last dim (DVE 2x mode).

Per-core n indexing: n = q*8 + 2r + h (q: 32 W blocks, r: 4 PE row
groups, h: psU partition half).  One chunk = (q, r): a single matmul
[K=32 zero-block-diag, M=128, N=1024] produces psU[64h+b, (k,o)] for
both h at once (tile_position=(32r, 0)).

Pipeline per chunk (engines overlap across chunks; per-chunk a-paths
are single-engine to avoid cross-engine ping-pong stalls):
  PE:    psU [128, 1024] = uZP-block^T @ WB-block          (2x 213 ns)
  ACT:   uh = psU -> bf16 into uhq slice                    (1038 ns)
  Pool-chunks (2 or 1 of the 4 per q, alternating):
    Pool: tmp = uh*v_bf; two k-halving folds -> th2 [128, 256]
    DVE:  aa = reduce_k(th2)                                (327 ns)
  DVE-chunks (the rest):
    DVE:  tmp = uh*v_bf (2x 594); th = k-fold (327); aa = reduce (594)
  per q: ONE batched Pool add blog_q += aq, then ACT exp directly on
  the logits (|b| <= ~0.8 so softmax needs no max shift).
phase2 (one q late, hides the softmax latency):
  DVE:   sm/rc/cc softmax tail; t2 = uhq * cc (ONE [128, 4096] op)
  PE:    psS += I2B^T @ t2 slices (s accumulation, 8x 213 ns)
psU bufs=3 and uhq bufs=6 keep PE/ACT running ahead through the
AllReduce windows.

Host-side layouts per core (W/u cast to bf16):
  WB  [32, 128, 1024]: WB[q, 16*p8+i, k*32+o] = W[q*8+p8, i, o*32+k]
  uB  [128, 2048]:     uB[16*p8+i, q*64+b] = u[b, q*8+p8, i]  (pass A)
  uZP [128, 4096]:     uZP[32r+16h+i, q*128+64h+b] = u[b, q*8+2r+h, i]
  I2B [128, 64]:       stacked 64x64 identities, bf16 (h/b merge)

Input DMAs are issued round-robin from the SP and ACT sequencers (uB
first so pass-A matmuls can chase the per-q W tiles; uZP/I2B last since
they are not needed until iteration 0), trimming the startup serial
segment before the first AllReduce.
"""

import numpy as np

B = 64
N_FULL = 2048
D_IN = 16
N_OUT = 32
D_OUT = 32
J = N_OUT * D_OUT  # 1024
N_CORES = 8
NL = N_FULL // N_CORES  # 256 local capsules
QB = NL // 8  # 32 q-blocks

_CACHE = {}


def _pack_inputs(u, W):
    """Shard along n and build per-core SBUF-friendly layouts (bf16)."""
    import ml_dtypes
    bf = ml_dtypes.bfloat16
    I2B = np.tile(np.eye(B, dtype=np.float32), (2, 1)).astype(bf)
    in_maps = []
    for c in range(N_CORES):
        ul = u[:, c * NL:(c + 1) * NL, :]          # [64, 256, 16]
        Wl = W[c * NL:(c + 1) * NL]                # [256, 16, 1024]
        # (k,o) layout: j' = k*32 + o
        Wko = np.ascontiguousarray(
            Wl.reshape(NL, D_IN, N_OUT, D_OUT).transpose(0, 1, 3, 2)
            .reshape(NL, D_IN, J))
        WB = np.ascontiguousarray(
            Wko.reshape(QB, 8, D_IN, J).reshape(QB, 128, J)).astype(bf)
        uB = np.ascontiguousarray(
            ul.reshape(B, QB, 8, D_IN).transpose(2, 3, 1, 0)
            .reshape(128, QB * B)).astype(bf)
        # uZP[32r+16h+i, q*128+64h'+b] = u[b, q*8+2r+h, i] iff h==h'
        un = ul.reshape(B, QB, 4, 2, D_IN)  # [b, q, r, h, i]
        Z = np.zeros((4, 2, D_IN, QB, 2, B), dtype=np.float32)
        for h in range(2):
            Z[:, h, :, :, h, :] = un[:, :, :, h, :].transpose(2, 3, 1, 0)
        uZP = Z.reshape(128, QB * 2 * B).astype(bf)
        in_maps.append({"WB": WB, "uB": uB, "uZP": uZP, "I2B": I2B})
    return in_maps


def _build_program():
    import concourse.bass as bass
    import concourse.tile as tile
    from concourse import bacc, mybir

    f32 = mybir.dt.float32
    bf16 = mybir.dt.bfloat16
    AF = mybir.ActivationFunctionType
    ALU = mybir.AluOpType
    AX = mybir.AxisListType

    nc = bacc.Bacc("TRN2", target_bir_lowering=False, debug=False,
                   num_devices=N_CORES)
    WB_d = nc.dram_tensor("WB", [QB, 128, J], bf16, kind="ExternalInput").ap()
    uB_d = nc.dram_tensor("uB", [128, QB * B], bf16, kind="ExternalInput").ap()
    uZP_d = nc.dram_tensor("uZP", [128, QB * 2 * B], bf16,
                           kind="ExternalInput").ap()
    I2B_d = nc.dram_tensor("I2B", [128, B], bf16, kind="ExternalInput").ap()
    v_d = nc.dram_tensor("v_out", [B, J], f32, kind="ExternalOutput").ap()

    with tile.TileContext(nc) as tc:
        with (
            tc.tile_pool(name="wpool", bufs=1) as wpool,
            tc.tile_pool(name="state", bufs=1) as state,
            tc.tile_pool(name="uhp", bufs=7) as uhp,
            tc.tile_pool(name="tmpp", bufs=1) as tmpp,
            tc.tile_pool(name="scratch", bufs=2) as scratch,
            tc.tile_pool(name="smalls", bufs=3) as smalls,
            tc.tile_pool(name="pU", bufs=3, space="PSUM") as pU,
            tc.tile_pool(name="pS", bufs=1, space="PSUM") as pS,
            tc.tile_pool(name="dram", bufs=2, space="DRAM") as dram,
        ):
            # --- load inputs: uB first (pass A gate), W chased by psA,
            # uZP/I2B last (not needed until iter 0).  Issue round-robin
            # over SP/ACT/DVE sequencers so issue time doesn't gate. ---
            issuers = [nc.sync, nc.scalar]
            uB_t = state.tile([128, QB * B], bf16, tag="uB")
            nc.sync.dma_start(uB_t[:], uB_d[:])
            w_tiles = []
            for q in range(QB):
                wt = wpool.tile([128, J], bf16, tag=f"w{q}", name=f"w{q}")
                issuers[q % 2].dma_start(wt[:], WB_d[q])
                w_tiles.append(wt)
            uZP_t = state.tile([128, QB * 2 * B], bf16, tag="uZP")
            nc.scalar.dma_start(uZP_t[:], uZP_d[:])
            I2B_t = state.tile([128, B], bf16, tag="I2B")
            nc.sync.dma_start(I2B_t[:], I2B_d[:])

            # logits blog[64h+b, (q*4+r)*32+o] for n = q*8+2r+h
            blog = state.tile([128, NL // 2 * N_OUT], bf16, tag="blog")
            nc.gpsimd.memset(blog[:], 0.0)
            v_t = state.tile([B, J], f32, tag="v")
            v_bf = state.tile([128, J], bf16, tag="v_bf")

            def ar_squash(merged_ps, scale, final=False):
                """merged [64,J] psum -> AllReduce -> squash -> v_t, v_bf4."""
                s_loc = scratch.tile([B, J], bf16, tag="st", bufs=1)
                nc.scalar.mul(s_loc[:], merged_ps[:], scale)
                bin_ = dram.tile([B, J], bf16, tag="bounce_in")
                bout = dram.tile([B, J], bf16, tag="bounce_out")
                nc.sync.dma_start(bin_[:], s_loc[:])
                nc.gpsimd.collective_compute(
                    "AllReduce", ALU.add,
                    replica_groups=[list(range(N_CORES))],
                    ins=[bin_.opt()], outs=[bout.opt()],
                )
                s_g = scratch.tile([B, J], bf16, tag="st2", bufs=1)
                nc.sync.dma_start(s_g[:], bout[:])
                # squash: v = s * sqrt(n2)/(1+n2);  (k,o): norm over k.
                # bf16 sq + one k-fold keep the chain ops in DVE 2x mode.
                sq = scratch.tile([B, J], bf16, tag="st3", bufs=1)
                nc.vector.tensor_mul(sq[:], s_g[:], s_g[:])
                sqh = scratch.tile([B, J // 2], bf16, tag="st4", bufs=1)
                nc.vector.tensor_add(sqh[:], sq[:, 0:512], sq[:, 512:1024])
                sqh2 = scratch.tile([B, J // 4], bf16, tag="st5", bufs=1)
                nc.vector.tensor_add(sqh2[:], sqh[:, 0:256], sqh[:, 256:512])
                n2 = smalls.tile([B, N_OUT], f32, tag="n2")
                nc.vector.reduce_sum(
                    n2[:], sqh2[:].rearrange("p (k o) -> p o k", o=N_OUT),
                    axis=AX.X)
                # Sqrt issued FIRST among the ACT ops: its LoadActFuncSet
                # (sqrt lives in a different act table than exp, 1283ns)
                # has no data deps, so issued ahead of the add it runs
                # during the collective instead of on the post-collective
                # critical chain.
                rt = smalls.tile([B, N_OUT], f32, tag="rt")
                nc.scalar.activation(rt[:], n2[:], AF.Sqrt)
                # the +1 on DVE so it runs in parallel with ACT's sqrt
                n2p1 = smalls.tile([B, N_OUT], f32, tag="n2p1")
                nc.vector.tensor_scalar_add(n2p1[:], n2[:], 1.0)
                rcp = smalls.tile([B, N_OUT], f32, tag="rcp")
                nc.vector.reciprocal(rcp[:], n2p1[:])
                if final:
                    scl = smalls.tile([B, N_OUT], f32, tag="scl")
                    nc.vector.tensor_mul(scl[:], rt[:], rcp[:])
                    nc.vector.tensor_mul(
                        v_t[:].rearrange("p (k o) -> p k o", o=N_OUT),
                        s_g[:].rearrange("p (k o) -> p k o", o=N_OUT),
                        scl[:].unsqueeze(1).broadcast_to([B, D_OUT, N_OUT]))
                else:
                    # intermediate iterations: bf16 scale produced natively
                    # (no cast hop) keeps the v mult in DVE 2x mode
                    scl_bf = smalls.tile([B, N_OUT], bf16, tag="sclbf")
                    nc.vector.tensor_mul(scl_bf[:], rt[:], rcp[:])
                    nc.vector.tensor_mul(
                        v_bf[0:B, :].rearrange("p (k o) -> p k o", o=N_OUT),
                        s_g[:].rearrange("p (k o) -> p k o", o=N_OUT),
                        scl_bf[:].unsqueeze(1).broadcast_to(
                            [B, D_OUT, N_OUT]))
                    nc.vector.tensor_copy(v_bf[B:2 * B, :], v_bf[0:B, :])

            # ---- pass A: s1 = (1/32) * sum_n uhat ----
            psA = pS.tile([B, J], f32, tag="psS", bufs=1)
            for q in range(QB):
                for jh in range(2):
                    nc.tensor.matmul(
                        psA[:, jh * 512:(jh + 1) * 512],
                        lhsT=uB_t[:, q * B:(q + 1) * B],
                        rhs=w_tiles[q][:, jh * 512:(jh + 1) * 512],
                        start=(q == 0), stop=(q == QB - 1))
            ar_squash(psA, 1.0 / N_OUT)

            # ---- passes B, C ----
            for it in range(2):
                psS = pS.tile([B, J], f32, tag="psS", bufs=1)
                uh_live = {}
                ee_live = {}

                def phase1(q):
                    # q=0 leans DVE: right after the AllReduce window the
                    # Pool chain can't start until v_bf lands, so the
                    # first q's extra Pool chunk only stalls DVE's tails
                    n_pool = 1 if q == 0 else (2 if q % 2 == 0 else 1)
                    # aq collects the q's 4 aa slices; one batched blog add
                    aq = smalls.tile([128, 4 * N_OUT], bf16, tag="aq")
                    # one uh tile for the whole q; evacs fill slices
                    uhq = uhp.tile([128, 4 * J], bf16, tag="uhq")

                    for r in range(4):
                        psU = pU.tile([128, J], f32, tag="psU", bufs=3)
                        for jh in range(2):
                            nc.tensor.matmul(
                                psU[:, jh * 512:(jh + 1) * 512],
                                lhsT=uZP_t[32 * r:32 * r + 32,
                                           q * 2 * B:(q + 1) * 2 * B],
                                rhs=w_tiles[q][32 * r:32 * r + 32,
                                               jh * 512:(jh + 1) * 512],
                                start=True, stop=True,
                                tile_position=(32 * r, 0))
                        nc.scalar.mul(uhq[:, r * J:(r + 1) * J], psU[:], 1.0)

                    th2all = tmpp.tile([128, J], bf16, tag="th2a",
                                        bufs=3)
                    # Pool-chunks first: start Pool's long chains early
                    for r in range(n_pool):
                        uh = uhq[:, r * J:(r + 1) * J]
                        tmp = tmpp.tile([128, J], bf16, tag="tmp", bufs=3)
                        nc.gpsimd.tensor_mul(tmp[:], uh, v_bf[:])
                        th = tmpp.tile([128, J // 2], bf16, tag="th", bufs=3)
                        nc.gpsimd.tensor_add(
                            th[:], tmp[:, 0:512], tmp[:, 512:1024])
                        nc.gpsimd.tensor_add(
                            th2all[:, r * 256:(r + 1) * 256],
                            th[:, 0:256], th[:, 256:512])
                    # DVE-chunks: self-contained on DVE; all chunks
                    # (both engines) deposit their [128,256] k-partials
                    # into ONE th2all tile so a single batched 3-op fold
                    # chain replaces four separate 3-op tails.
                    for r in range(n_pool, 4):
                        uh = uhq[:, r * J:(r + 1) * J]
                        tmp = tmpp.tile([128, J], bf16, tag="tmp", bufs=3)
                        nc.vector.tensor_mul(tmp[:], uh, v_bf[:])
                        th = tmpp.tile([128, J // 2], bf16, tag="thd",
                                       bufs=3)
                        nc.vector.tensor_add(
                            th[:], tmp[:, 0:512], tmp[:, 512:1024])
                        nc.vector.tensor_add(
                            th2all[:, r * 256:(r + 1) * 256],
                            th[:, 0:256], th[:, 256:512])
                    # batched tail: three r-blocked k-halving folds
                    # [128, (4,256)] -> [128, (4,32)] = aq, all DVE 2x
                    f3 = tmpp.tile([128, J // 2], bf16, tag="f3", bufs=2)
                    iv = th2all[:].rearrange("p (r h x) -> p r h x",
                                             r=4, h=2, x=128)
                    nc.vector.tensor_add(
                        f3[:].rearrange("p (r x) -> p r x", r=4),
                        iv[:, :, 0], iv[:, :, 1])
                    f4 = tmpp.tile([128, J // 4], bf16, tag="f4", bufs=2)
                    iv = f3[:].rearrange("p (r h x) -> p r h x",
                                         r=4, h=2, x=64)
                    nc.vector.tensor_add(
                        f4[:].rearrange("p (r x) -> p r x", r=4),
                        iv[:, :, 0], iv[:, :, 1])
                    iv = f4[:].rearrange("p (r h x) -> p r h x",
                                         r=4, h=2, x=32)
                    nc.vector.tensor_add(
                        aq[:].rearrange("p (r x) -> p r x", r=4),
                        iv[:, :, 0], iv[:, :, 1])
                    # single batched logits update for the whole q,
                    # on DVE: aq's writer is the DVE fold tail, so this
                    # avoids a DVE->Pool->ACT round-trip on the exp chain
                    bq = blog[:, q * 4 * N_OUT:(q + 1) * 4 * N_OUT]
                    nc.vector.tensor_add(bq, bq, aq[:])
                    # exp straight off the logits: |b| <= ~0.8 so the
                    # softmax max-shift is unnecessary (shift invariance)
                    ee = smalls.tile([128, 4 * N_OUT], f32, tag="ee")
                    nc.scalar.activation(ee[:], bq, AF.Exp)
                    ee_live[q] = ee
                    uh_live[q] = uhq

                ee_live = {}

                def a_chain(eng, uhq, aq, r0, nr):
                    """Batched agreement chain over nr r-blocks starting
                    at r0: tmp = uh*v, 5 k-halving folds -> aq slice."""
                    tmp = tmpp.tile([128, nr * J], bf16, tag=f"tmp{nr}",
                                    name=f"tmp{nr}", bufs=2 if nr == 2 else 1)
                    eng.tensor_mul(tmp[:], uhq[:, r0 * J:(r0 + nr) * J],
                                   v_bf4[:, 0:nr * J])
                    cur = tmp
                    width = J
                    for lvl in range(5):
                        width //= 2
                        if lvl == 4:
                            nxt_ap = aq[:, r0 * N_OUT:(r0 + nr) * N_OUT]
                        else:
                            nxt = tmpp.tile([128, nr * width], bf16,
                                            tag=f"fold{nr}_{lvl}",
                                            name=f"fold{nr}_{lvl}", bufs=2)
                            nxt_ap = nxt[:]
                        iv = cur[:].rearrange("p (r h x) -> p r h x",
                                              r=nr, h=2, x=width)
                        eng.tensor_add(
                            nxt_ap.rearrange("p (r x) -> p r x", r=nr),
                            iv[:, :, 0], iv[:, :, 1])
                        if lvl < 4:
                            cur = nxt

                def phase1(q):
                    eng = nc.gpsimd if q in POOL_QS else nc.vector
                    # one uh tile for the whole q; evacs fill slices
                    uhq = uhp.tile([128, 4 * J], bf16, tag="uhq")
                    aq = smalls.tile([128, 4 * N_OUT], bf16, tag="aq")
                    for r in range(4):
                        psU = pU.tile([128, J], f32, tag="psU", bufs=3)
                        for jh in range(2):
                            nc.tensor.matmul(
                                psU[:, jh * 512:(jh + 1) * 512],
                                lhsT=uZP_t[32 * r:32 * r + 32,
                                           q * 2 * B:(q + 1) * 2 * B],
                                rhs=w_tiles[q][32 * r:32 * r + 32,
                                               jh * 512:(jh + 1) * 512],
                                start=True, stop=True,
                                tile_position=(32 * r, 0))
                        nc.scalar.mul(uhq[:, r * J:(r + 1) * J], psU[:], 1.0)
                        # DVE-q: launch the half-chain as soon as its two
                        # evacs are in, overlapping the remaining evacs.
                        if eng is nc.vector and r == 1:
                            a_chain(eng, uhq, aq, 0, 2)
                    if eng is nc.vector:
                        a_chain(eng, uhq, aq, 2, 2)
                    else:
                        a_chain(eng, uhq, aq, 0, 4)
                    # single batched logits update
                    bq = blog[:, q * 4 * N_OUT:(q + 1) * 4 * N_OUT]
                    eng.tensor_add(bq, bq, aq[:])
                    # exp straight off the logits: |b| <= ~0.8 so the
                    # softmax max-shift is unnecessary (shift invariance)
                    ee = smalls.tile([128, 4 * N_OUT], f32, tag="ee")
                    nc.scalar.activation(ee[:], bq, AF.Exp)
                    ee_live[q] = ee
                    uh_live[q] = uhq

                def phase2(q, last):
                    # softmax tail (exp already issued in phase1), t2, merge
                    uhq = uh_live.pop(q)
                    ee = ee_live.pop(q)
                    sm = smalls.tile([128, 4], f32, tag="sm")
                    nc.vector.reduce_sum(
                        sm[:], ee[:].rearrange("p (r o) -> p r o", o=N_OUT),
                        axis=AX.X)
                    rc = smalls.tile([128, 4], f32, tag="rc")
                    nc.vector.reciprocal(rc[:], sm[:])
                    cc = smalls.tile([128, 4 * N_OUT], bf16, tag="cc")
                    nc.vector.tensor_tensor(
                        cc[:].rearrange("p (r o) -> p r o", o=N_OUT),
                        ee[:].rearrange("p (r o) -> p r o", o=N_OUT),
                        rc[:].unsqueeze(2).broadcast_to([128, 4, N_OUT]),
                        op=ALU.mult)
                    if not last:
                        # batched t2 = uh * c for all 4 r at once (bf16 2x)
                        t2 = tmpp.tile([128, 4 * J], bf16, tag="t2", bufs=2)
                        nc.vector.tensor_tensor(
                            t2[:].rearrange("p (r k o) -> p r k o",
                                            k=D_OUT, o=N_OUT),
                            uhq[:].rearrange("p (r k o) -> p r k o",
                                             k=D_OUT, o=N_OUT),
                            cc[:].rearrange("p (r o) -> p r o", o=N_OUT)
                            .unsqueeze(2).broadcast_to(
                                [128, 4, D_OUT, N_OUT]),
                            op=ALU.mult)
                        for sl in range(8):
                            nc.tensor.matmul(
                                psS[:, (sl % 2) * 512:(sl % 2 + 1) * 512],
                                lhsT=I2B_t[:],
                                rhs=t2[:, sl * 512:(sl + 1) * 512],
                                start=(q == 0 and sl < 2),
                                stop=False)
                    else:
                        # final q: r-sliced t2 with interleaved psS pairs
                        # so the s accumulation (gating the AllReduce)
                        # completes ~1.6us earlier
                        for r4 in range(4):
                            t2s = tmpp.tile([128, J], bf16, tag="t2s",
                                            name="t2s", bufs=2)
                            nc.vector.tensor_tensor(
                                t2s[:].rearrange("p (k o) -> p k o",
                                                 o=N_OUT),
                                uhq[:, r4 * J:(r4 + 1) * J]
                                .rearrange("p (k o) -> p k o", o=N_OUT),
                                cc[:, r4 * N_OUT:(r4 + 1) * N_OUT]
                                .unsqueeze(1).broadcast_to(
                                    [128, D_OUT, N_OUT]),
                                op=ALU.mult)
                            for jh in range(2):
                                nc.tensor.matmul(
                                    psS[:, jh * 512:(jh + 1) * 512],
                                    lhsT=I2B_t[:],
                                    rhs=t2s[:, jh * 512:(jh + 1) * 512],
                                    start=False,
                                    stop=(r4 == 3))

                for q in range(QB):
                    phase1(q)
                    if q >= 1:
                        phase2(q - 1, last=False)
                phase2(QB - 1, last=True)
                ar_squash(psS, 1.0, final=(it == 1))

            nc.sync.dma_start(v_d[:], v_t[:])

    nc.compile()
    return nc


def _get_program():
    if "nc" not in _CACHE:
        _CACHE["nc"] = _build_program()
    return _CACHE["nc"]


def kernel(u, W):
    from concourse.bass_utils import run_bass_kernel_spmd

    nc = _get_program()
    in_maps = _pack_inputs(np.asarray(u, np.float32), np.asarray(W, np.float32))
    res = run_bass_kernel_spmd(nc, in_maps, list(range(N_CORES)))
    v = res.results[0]["v_out"]
    # (k,o) layout -> [b, o, k]
    return np.ascontiguousarray(
        v.reshape(B, D_OUT, N_OUT).transpose(0, 2, 1))


# revision 43
# speedup vs baseline: 1.0040x; 1.0040x over previous
"""Trainium2 Bass kernel for capsule-network dynamic routing.

Problem: u [64, 2048, 16], W [2048, 16, 1024] ->
  uhat = einsum('bni,nij->bnj', u, W)  (viewed [B, N, 32, 32])
  3 routing iterations (softmax over out-caps, squash) -> v [64, 32, 32]

Sharding: n (input capsules) split across 8 cores, 256 per core.
W slice stays SBUF-resident (bf16); uhat is recomputed on the PE each
routing pass (never materialized to HBM).  The per-iteration s-reduction
([64, 1024] partial sums) is AllReduced across cores.

Layout: j is stored k-major (j' = k*32 + o, "(k,o)") so the c-weighting
(t2 = uh * c) broadcasts c over k with a packed last dim (DVE 2x mode).

Per-core n indexing: n = q*8 + 2r + h (q: 32 W blocks, r: 4 PE row
groups, h: psU partition half).  One chunk = (q, r): a single matmul
[K=32 zero-block-diag, M=128, N=1024] produces psU[64h+b, (k,o)] for
both h at once (tile_position=(32r, 0)).

Pipeline per chunk (engines overlap across chunks; per-chunk a-paths
are single-engine to avoid cross-engine ping-pong stalls):
  PE:    psU [128, 1024] = uZP-block^T @ WB-block          (2x 213 ns)
  ACT:   uh = psU -> bf16 into uhq slice                    (1038 ns)
  Pool-chunks (2 or 1 of the 4 per q, alternating):
    Pool: tmp = uh*v_bf; two k-halving folds -> th2 [128, 256]
    DVE:  aa = reduce_k(th2)                                (327 ns)
  DVE-chunks (the rest):
    DVE:  tmp = uh*v_bf (2x 594); th = k-fold (327); aa = reduce (594)
  per q: ONE batched Pool add blog_q += aq, then ACT exp directly on
  the logits (|b| <= ~0.8 so softmax needs no max shift).
phase2 (one q late, hides the softmax latency):
  DVE:   sm/rc/cc softmax tail; t2 = uhq * cc (ONE [128, 4096] op)
  PE:    psS += I2B^T @ t2 slices (s accumulation, 8x 213 ns)
psU bufs=3 and uhq bufs=6 keep PE/ACT running ahead through the
AllReduce windows.

Host-side layouts per core (W/u cast to bf16):
  WB  [32, 128, 1024]: WB[q, 16*p8+i, k*32+o] = W[q*8+p8, i, o*32+k]
  uB  [128, 2048]:     uB[16*p8+i, q*64+b] = u[b, q*8+p8, i]  (pass A)
  uZP [128, 4096]:     uZP[32r+16h+i, q*128+64h+b] = u[b, q*8+2r+h, i]
  I2B [128, 64]:       stacked 64x64 identities, bf16 (h/b merge)

Input DMAs are issued round-robin from the SP and ACT sequencers (uB
first so pass-A matmuls can chase the per-q W tiles; uZP/I2B last since
they are not needed until iteration 0), trimming the startup serial
segment before the first AllReduce.
"""

import numpy as np

B = 64
N_FULL = 2048
D_IN = 16
N_OUT = 32
D_OUT = 32
J = N_OUT * D_OUT  # 1024
N_CORES = 8
NL = N_FULL // N_CORES  # 256 local capsules
QB = NL // 8  # 32 q-blocks

_CACHE = {}


def _pack_inputs(u, W):
    """Shard along n and build per-core SBUF-friendly layouts (bf16)."""
    import ml_dtypes
    bf = ml_dtypes.bfloat16
    I2B = np.tile(np.eye(B, dtype=np.float32), (2, 1)).astype(bf)
    in_maps = []
    for c in range(N_CORES):
        ul = u[:, c * NL:(c + 1) * NL, :]          # [64, 256, 16]
        Wl = W[c * NL:(c + 1) * NL]                # [256, 16, 1024]
        # (k,o) layout: j' = k*32 + o
        Wko = np.ascontiguousarray(
            Wl.reshape(NL, D_IN, N_OUT, D_OUT).transpose(0, 1, 3, 2)
            .reshape(NL, D_IN, J))
        WB = np.ascontiguousarray(
            Wko.reshape(QB, 8, D_IN, J).reshape(QB, 128, J)).astype(bf)
        uB = np.ascontiguousarray(
            ul.reshape(B, QB, 8, D_IN).transpose(2, 3, 1, 0)
            .reshape(128, QB * B)).astype(bf)
        # uZP[32r+16h+i, q*128+64h'+b] = u[b, q*8+2r+h, i] iff h==h'
        un = ul.reshape(B, QB, 4, 2, D_IN)  # [b, q, r, h, i]
        Z = np.zeros((4, 2, D_IN, QB, 2, B), dtype=np.float32)
        for h in range(2):
            Z[:, h, :, :, h, :] = un[:, :, :, h, :].transpose(2, 3, 1, 0)
        uZP = Z.reshape(128, QB * 2 * B).astype(bf)
        in_maps.append({"WB": WB, "uB": uB, "uZP": uZP, "I2B": I2B})
    return in_maps


def _build_program():
    import concourse.bass as bass
    import concourse.tile as tile
    from concourse import bacc, mybir

    f32 = mybir.dt.float32
    bf16 = mybir.dt.bfloat16
    AF = mybir.ActivationFunctionType
    ALU = mybir.AluOpType
    AX = mybir.AxisListType

    nc = bacc.Bacc("TRN2", target_bir_lowering=False, debug=False,
                   num_devices=N_CORES)
    WB_d = nc.dram_tensor("WB", [QB, 128, J], bf16, kind="ExternalInput").ap()
    uB_d = nc.dram_tensor("uB", [128, QB * B], bf16, kind="ExternalInput").ap()
    uZP_d = nc.dram_tensor("uZP", [128, QB * 2 * B], bf16,
                           kind="ExternalInput").ap()
    I2B_d = nc.dram_tensor("I2B", [128, B], bf16, kind="ExternalInput").ap()
    v_d = nc.dram_tensor("v_out", [B, J], f32, kind="ExternalOutput").ap()

    with tile.TileContext(nc) as tc:
        with (
            tc.tile_pool(name="wpool", bufs=1) as wpool,
            tc.tile_pool(name="state", bufs=1) as state,
            tc.tile_pool(name="uhp", bufs=7) as uhp,
            tc.tile_pool(name="tmpp", bufs=1) as tmpp,
            tc.tile_pool(name="scratch", bufs=2) as scratch,
            tc.tile_pool(name="smalls", bufs=3) as smalls,
            tc.tile_pool(name="pU", bufs=3, space="PSUM") as pU,
            tc.tile_pool(name="pS", bufs=1, space="PSUM") as pS,
            tc.tile_pool(name="dram", bufs=2, space="DRAM") as dram,
        ):
            # --- load inputs: uB first (pass A gate), W chased by psA,
            # uZP/I2B last (not needed until iter 0).  Issue round-robin
            # over SP/ACT/DVE sequencers so issue time doesn't gate. ---
            issuers = [nc.sync, nc.scalar]
            uB_t = state.tile([128, QB * B], bf16, tag="uB")
            nc.sync.dma_start(uB_t[:], uB_d[:])
            w_tiles = []
            for q in range(QB):
                wt = wpool.tile([128, J], bf16, tag=f"w{q}", name=f"w{q}")
                issuers[q % 2].dma_start(wt[:], WB_d[q])
                w_tiles.append(wt)
            uZP_t = state.tile([128, QB * 2 * B], bf16, tag="uZP")
            nc.scalar.dma_start(uZP_t[:], uZP_d[:])
            I2B_t = state.tile([128, B], bf16, tag="I2B")
            nc.sync.dma_start(I2B_t[:], I2B_d[:])

            # logits blog[64h+b, (q*4+r)*32+o] for n = q*8+2r+h
            blog = state.tile([128, NL // 2 * N_OUT], bf16, tag="blog")
            nc.gpsimd.memset(blog[:], 0.0)
            v_t = state.tile([B, J], f32, tag="v")
            v_bf = state.tile([128, J], bf16, tag="v_bf")

            def ar_squash(merged_ps, scale, final=False):
                """merged [64,J] psum -> AllReduce -> squash -> v_t, v_bf4."""
                s_loc = scratch.tile([B, J], bf16, tag="st", bufs=1)
                nc.scalar.mul(s_loc[:], merged_ps[:], scale)
                bin_ = dram.tile([B, J], bf16, tag="bounce_in")
                bout = dram.tile([B, J], bf16, tag="bounce_out")
                nc.sync.dma_start(bin_[:], s_loc[:])
                nc.gpsimd.collective_compute(
                    "AllReduce", ALU.add,
                    replica_groups=[list(range(N_CORES))],
                    ins=[bin_.opt()], outs=[bout.opt()],
                )
                s_g = scratch.tile([B, J], bf16, tag="st2", bufs=1)
                nc.sync.dma_start(s_g[:], bout[:])
                # squash: v = s * sqrt(n2)/(1+n2);  (k,o): norm over k.
                # bf16 sq + one k-fold keep the chain ops in DVE 2x mode.
                sq = scratch.tile([B, J], bf16, tag="st3", bufs=1)
                nc.vector.tensor_mul(sq[:], s_g[:], s_g[:])
                sqh = scratch.tile([B, J // 2], bf16, tag="st4", bufs=1)
                nc.vector.tensor_add(sqh[:], sq[:, 0:512], sq[:, 512:1024])
                sqh2 = scratch.tile([B, J // 4], bf16, tag="st5", bufs=1)
                nc.vector.tensor_add(sqh2[:], sqh[:, 0:256], sqh[:, 256:512])
                n2 = smalls.tile([B, N_OUT], f32, tag="n2")
                nc.vector.reduce_sum(
                    n2[:], sqh2[:].rearrange("p (k o) -> p o k", o=N_OUT),
                    axis=AX.X)
                # Sqrt issued FIRST among the ACT ops: its LoadActFuncSet
                # (sqrt lives in a different act table than exp, 1283ns)
                # has no data deps, so issued ahead of the add it runs
                # during the collective instead of on the post-collective
                # critical chain.
                rt = smalls.tile([B, N_OUT], f32, tag="rt")
                nc.scalar.activation(rt[:], n2[:], AF.Sqrt)
                # the +1 on DVE so it runs in parallel with ACT's sqrt
                n2p1 = smalls.tile([B, N_OUT], f32, tag="n2p1")
                nc.vector.tensor_scalar_add(n2p1[:], n2[:], 1.0)
                rcp = smalls.tile([B, N_OUT], f32, tag="rcp")
                nc.vector.reciprocal(rcp[:], n2p1[:])
                if final:
                    scl = smalls.tile([B, N_OUT], f32, tag="scl")
                    nc.vector.tensor_mul(scl[:], rt[:], rcp[:])
                    nc.vector.tensor_mul(
                        v_t[:].rearrange("p (k o) -> p k o", o=N_OUT),
                        s_g[:].rearrange("p (k o) -> p k o", o=N_OUT),
                        scl[:].unsqueeze(1).broadcast_to([B, D_OUT, N_OUT]))
                else:
                    # intermediate iterations: bf16 scale produced natively
                    # (no cast hop) keeps the v mult in DVE 2x mode
                    scl_bf = smalls.tile([B, N_OUT], bf16, tag="sclbf")
                    nc.vector.tensor_mul(scl_bf[:], rt[:], rcp[:])
                    nc.vector.tensor_mul(
                        v_bf[0:B, :].rearrange("p (k o) -> p k o", o=N_OUT),
                        s_g[:].rearrange("p (k o) -> p k o", o=N_OUT),
                        scl_bf[:].unsqueeze(1).broadcast_to(
                            [B, D_OUT, N_OUT]))
                    nc.vector.tensor_copy(v_bf[B:2 * B, :], v_bf[0:B, :])

            # ---- pass A: s1 = (1/32) * sum_n uhat ----
            psA = pS.tile([B, J], f32, tag="psS", bufs=1)
            for q in range(QB):
                for jh in range(2):
                    nc.tensor.matmul(
                        psA[:, jh * 512:(jh + 1) * 512],
                        lhsT=uB_t[:, q * B:(q + 1) * B],
                        rhs=w_tiles[q][:, jh * 512:(jh + 1) * 512],
                        start=(q == 0), stop=(q == QB - 1))
            ar_squash(psA, 1.0 / N_OUT)

            # ---- passes B, C ----
            for it in range(2):
                psS = pS.tile([B, J], f32, tag="psS", bufs=1)
                uh_live = {}
                ee_live = {}

                def phase1(q):
                    # q=0 leans DVE: right after the AllReduce window the
                    # Pool chain can't start until v_bf lands, so the
                    # first q's extra Pool chunk only stalls DVE's tails
                    n_pool = 1 if q == 0 else (2 if q % 2 == 0 else 1)
                    # aq collects the q's 4 aa slices; one batched blog add
                    aq = smalls.tile([128, 4 * N_OUT], bf16, tag="aq")
                    # one uh tile for the whole q; evacs fill slices
                    uhq = uhp.tile([128, 4 * J], bf16, tag="uhq")

                    for r in range(4):
                        psU = pU.tile([128, J], f32, tag="psU", bufs=3)
                        for jh in range(2):
                            nc.tensor.matmul(
                                psU[:, jh * 512:(jh + 1) * 512],
                                lhsT=uZP_t[32 * r:32 * r + 32,
                                           q * 2 * B:(q + 1) * 2 * B],
                                rhs=w_tiles[q][32 * r:32 * r + 32,
                                               jh * 512:(jh + 1) * 512],
                                start=True, stop=True,
                                tile_position=(32 * r, 0))
                        nc.scalar.mul(uhq[:, r * J:(r + 1) * J], psU[:], 1.0)

                    th2all = tmpp.tile([128, J], bf16, tag="th2a",
                                        bufs=3)
                    # Pool-chunks first: start Pool's long chains early
                    for r in range(n_pool):
                        uh = uhq[:, r * J:(r + 1) * J]
                        tmp = tmpp.tile([128, J], bf16, tag="tmp", bufs=3)
                        nc.gpsimd.tensor_mul(tmp[:], uh, v_bf[:])
                        th = tmpp.tile([128, J // 2], bf16, tag="th", bufs=3)
                        nc.gpsimd.tensor_add(
                            th[:], tmp[:, 0:512], tmp[:, 512:1024])
                        nc.gpsimd.tensor_add(
                            th2all[:, r * 256:(r + 1) * 256],
                            th[:, 0:256], th[:, 256:512])
                    # DVE-chunks: r2/r3 always both DVE -> their tmp
                    # mults land in one tmpall tile and the th/th2 folds
                    # run pair-wide (saves two op overheads per q); an r1
                    # DVE chunk (odd q) keeps the classic 3-op chain.
                    if n_pool == 1:
                        uh = uhq[:, J:2 * J]
                        tmp = tmpp.tile([128, J], bf16, tag="tmp", bufs=3)
                        nc.vector.tensor_mul(tmp[:], uh, v_bf[:])
                        th = tmpp.tile([128, J // 2], bf16, tag="thd",
                                       bufs=3)
                        nc.vector.tensor_add(
                            th[:], tmp[:, 0:512], tmp[:, 512:1024])
                        nc.vector.tensor_add(
                            th2all[:, 256:512], th[:, 0:256],
                            th[:, 256:512])
                    tmpall = tmpp.tile([128, 2 * J], bf16, tag="tmpd",
                                       bufs=1)
                    for i in range(2):
                        nc.vector.tensor_mul(
                            tmpall[:, i * J:(i + 1) * J],
                            uhq[:, (2 + i) * J:(3 + i) * J], v_bf[:])
                    thp = tmpp.tile([128, J], bf16, tag="thp", bufs=1)
                    iv = tmpall[:].rearrange("p (c h x) -> p c h x",
                                             c=2, h=2, x=512)
                    nc.vector.tensor_add(
                        thp[:].rearrange("p (c x) -> p c x", c=2),
                        iv[:, :, 0], iv[:, :, 1])
                    iv = thp[:].rearrange("p (c h x) -> p c h x",
                                          c=2, h=2, x=256)
                    nc.vector.tensor_add(
                        th2all[:, 512:1024].rearrange(
                            "p (c x) -> p c x", c=2),
                        iv[:, :, 0], iv[:, :, 1])
                    # batched tail: three r-blocked k-halving folds
                    # [128, (4,256)] -> [128, (4,32)] = aq, all DVE 2x
                    f3 = tmpp.tile([128, J // 2], bf16, tag="f3", bufs=2)
                    iv = th2all[:].rearrange("p (r h x) -> p r h x",
                                             r=4, h=2, x=128)
                    nc.vector.tensor_add(
                        f3[:].rearrange("p (r x) -> p r x", r=4),
                        iv[:, :, 0], iv[:, :, 1])
                    f4 = tmpp.tile([128, J // 4], bf16, tag="f4", bufs=2)
                    iv = f3[:].rearrange("p (r h x) -> p r h x",
                                         r=4, h=2, x=64)
                    nc.vector.tensor_add(
                        f4[:].rearrange("p (r x) -> p r x", r=4),
                        iv[:, :, 0], iv[:, :, 1])
                    iv = f4[:].rearrange("p (r h x) -> p r h x",
                                         r=4, h=2, x=32)
                    nc.vector.tensor_add(
                        aq[:].rearrange("p (r x) -> p r x", r=4),
                        iv[:, :, 0], iv[:, :, 1])
                    # single batched logits update for the whole q,
                    # on DVE: aq's writer is the DVE fold tail, so this
                    # avoids a DVE->Pool->ACT round-trip on the exp chain
                    bq = blog[:, q * 4 * N_OUT:(q + 1) * 4 * N_OUT]
                    nc.vector.tensor_add(bq, bq, aq[:])
                    # exp straight off the logits: |b| <= ~0.8 so the
                    # softmax max-shift is unnecessary (shift invariance)
                    ee = smalls.tile([128, 4 * N_OUT], f32, tag="ee")
                    nc.scalar.activation(ee[:], bq, AF.Exp)
                    ee_live[q] = ee
                    uh_live[q] = uhq

                ee_live = {}

                def a_chain(eng, uhq, aq, r0, nr):
                    """Batched agreement chain over nr r-blocks starting
                    at r0: tmp = uh*v, 5 k-halving folds -> aq slice."""
                    tmp = tmpp.tile([128, nr * J], bf16, tag=f"tmp{nr}",
                                    name=f"tmp{nr}", bufs=2 if nr == 2 else 1)
                    eng.tensor_mul(tmp[:], uhq[:, r0 * J:(r0 + nr) * J],
                                   v_bf4[:, 0:nr * J])
                    cur = tmp
                    width = J
                    for lvl in range(5):
                        width //= 2
                        if lvl == 4:
                            nxt_ap = aq[:, r0 * N_OUT:(r0 + nr) * N_OUT]
                        else:
                            nxt = tmpp.tile([128, nr * width], bf16,
                                            tag=f"fold{nr}_{lvl}",
                                            name=f"fold{nr}_{lvl}", bufs=2)
                            nxt_ap = nxt[:]
                        iv = cur[:].rearrange("p (r h x) -> p r h x",
                                              r=nr, h=2, x=width)
                        eng.tensor_add(
                            nxt_ap.rearrange("p (r x) -> p r x", r=nr),
                            iv[:, :, 0], iv[:, :, 1])
                        if lvl < 4:
                            cur = nxt

                def phase1(q):
                    eng = nc.gpsimd if q in POOL_QS else nc.vector
                    # one uh tile for the whole q; evacs fill slices
                    uhq = uhp.tile([128, 4 * J], bf16, tag="uhq")
                    aq = smalls.tile([128, 4 * N_OUT], bf16, tag="aq")
                    for r in range(4):
                        psU = pU.tile([128, J], f32, tag="psU", bufs=3)
                        for jh in range(2):
                            nc.tensor.matmul(
                                psU[:, jh * 512:(jh + 1) * 512],
                                lhsT=uZP_t[32 * r:32 * r + 32,
                                           q * 2 * B:(q + 1) * 2 * B],
                                rhs=w_tiles[q][32 * r:32 * r + 32,
                                               jh * 512:(jh + 1) * 512],
                                start=True, stop=True,
                                tile_position=(32 * r, 0))
                        nc.scalar.mul(uhq[:, r * J:(r + 1) * J], psU[:], 1.0)
                        # DVE-q: launch the half-chain as soon as its two
                        # evacs are in, overlapping the remaining evacs.
                        if eng is nc.vector and r == 1:
                            a_chain(eng, uhq, aq, 0, 2)
                    if eng is nc.vector:
                        a_chain(eng, uhq, aq, 2, 2)
                    else:
                        a_chain(eng, uhq, aq, 0, 4)
                    # single batched logits update
                    bq = blog[:, q * 4 * N_OUT:(q + 1) * 4 * N_OUT]
                    eng.tensor_add(bq, bq, aq[:])
                    # exp straight off the logits: |b| <= ~0.8 so the
                    # softmax max-shift is unnecessary (shift invariance)
                    ee = smalls.tile([128, 4 * N_OUT], f32, tag="ee")
                    nc.scalar.activation(ee[:], bq, AF.Exp)
                    ee_live[q] = ee
                    uh_live[q] = uhq

                def phase2(q, last):
                    # softmax tail (exp already issued in phase1), t2, merge
                    uhq = uh_live.pop(q)
                    ee = ee_live.pop(q)
                    sm = smalls.tile([128, 4], f32, tag="sm")
                    nc.vector.reduce_sum(
                        sm[:], ee[:].rearrange("p (r o) -> p r o", o=N_OUT),
                        axis=AX.X)
                    rc = smalls.tile([128, 4], f32, tag="rc")
                    nc.vector.reciprocal(rc[:], sm[:])
                    cc = smalls.tile([128, 4 * N_OUT], bf16, tag="cc")
                    nc.vector.tensor_tensor(
                        cc[:].rearrange("p (r o) -> p r o", o=N_OUT),
                        ee[:].rearrange("p (r o) -> p r o", o=N_OUT),
                        rc[:].unsqueeze(2).broadcast_to([128, 4, N_OUT]),
                        op=ALU.mult)
                    if not last:
                        # batched t2 = uh * c for all 4 r at once (bf16 2x)
                        t2 = tmpp.tile([128, 4 * J], bf16, tag="t2", bufs=2)
                        nc.vector.tensor_tensor(
                            t2[:].rearrange("p (r k o) -> p r k o",
                                            k=D_OUT, o=N_OUT),
                            uhq[:].rearrange("p (r k o) -> p r k o",
                                             k=D_OUT, o=N_OUT),
                            cc[:].rearrange("p (r o) -> p r o", o=N_OUT)
                            .unsqueeze(2).broadcast_to(
                                [128, 4, D_OUT, N_OUT]),
                            op=ALU.mult)
                        for sl in range(8):
                            nc.tensor.matmul(
                                psS[:, (sl % 2) * 512:(sl % 2 + 1) * 512],
                                lhsT=I2B_t[:],
                                rhs=t2[:, sl * 512:(sl + 1) * 512],
                                start=(q == 0 and sl < 2),
                                stop=False)
                    else:
                        # final q: r-sliced t2 with interleaved psS pairs
                        # so the s accumulation (gating the AllReduce)
                        # completes ~1.6us earlier
                        for r4 in range(4):
                            t2s = tmpp.tile([128, J], bf16, tag="t2s",
                                            name="t2s", bufs=2)
                            nc.vector.tensor_tensor(
                                t2s[:].rearrange("p (k o) -> p k o",
                                                 o=N_OUT),
                                uhq[:, r4 * J:(r4 + 1) * J]
                                .rearrange("p (k o) -> p k o", o=N_OUT),
                                cc[:, r4 * N_OUT:(r4 + 1) * N_OUT]
                                .unsqueeze(1).broadcast_to(
                                    [128, D_OUT, N_OUT]),
                                op=ALU.mult)
                            for jh in range(2):
                                nc.tensor.matmul(
                                    psS[:, jh * 512:(jh + 1) * 512],
                                    lhsT=I2B_t[:],
                                    rhs=t2s[:, jh * 512:(jh + 1) * 512],
                                    start=False,
                                    stop=(r4 == 3))

                for q in range(QB):
                    phase1(q)
                    if q >= 1:
                        phase2(q - 1, last=False)
                phase2(QB - 1, last=True)
                ar_squash(psS, 1.0, final=(it == 1))

            nc.sync.dma_start(v_d[:], v_t[:])

    nc.compile()
    return nc


def _get_program():
    if "nc" not in _CACHE:
        _CACHE["nc"] = _build_program()
    return _CACHE["nc"]


def kernel(u, W):
    from concourse.bass_utils import run_bass_kernel_spmd

    nc = _get_program()
    in_maps = _pack_inputs(np.asarray(u, np.float32), np.asarray(W, np.float32))
    res = run_bass_kernel_spmd(nc, in_maps, list(range(N_CORES)))
    v = res.results[0]["v_out"]
    # (k,o) layout -> [b, o, k]
    return np.ascontiguousarray(
        v.reshape(B, D_OUT, N_OUT).transpose(0, 2, 1))


# revision 48
# speedup vs baseline: 1.0097x; 1.0057x over previous
"""Trainium2 Bass kernel for capsule-network dynamic routing.

Problem: u [64, 2048, 16], W [2048, 16, 1024] ->
  uhat = einsum('bni,nij->bnj', u, W)  (viewed [B, N, 32, 32])
  3 routing iterations (softmax over out-caps, squash) -> v [64, 32, 32]

Sharding: n (input capsules) split across 8 cores, 256 per core.
W slice stays SBUF-resident (bf16); uhat is recomputed on the PE each
routing pass (never materialized to HBM).  The per-iteration s-reduction
([64, 1024] partial sums) is AllReduced across cores.

Layout: j is stored k-major (j' = k*32 + o, "(k,o)") so the c-weighting
(t2 = uh * c) broadcasts c over k with a packed last dim (DVE 2x mode).

Per-core n indexing: n = q*8 + 2r + h (q: 32 W blocks, r: 4 PE row
groups, h: psU partition half).  One chunk = (q, r): a single matmul
[K=32 zero-block-diag, M=128, N=1024] produces psU[64h+b, (k,o)] for
both h at once (tile_position=(32r, 0)).

Pipeline per chunk (engines overlap across chunks; per-chunk a-paths
are single-engine to avoid cross-engine ping-pong stalls):
  PE:    psU [128, 1024] = uZP-block^T @ WB-block          (2x 213 ns)
  ACT:   uh = psU -> bf16 into uhq slice                    (1038 ns)
  Pool-chunks (2 or 1 of the 4 per q, alternating):
    Pool: tmp = uh*v_bf; two k-halving folds -> th2 [128, 256]
    DVE:  aa = reduce_k(th2)                                (327 ns)
  DVE-chunks (the rest):
    DVE:  tmp = uh*v_bf (2x 594); th = k-fold (327); aa = reduce (594)
  per q: ONE batched Pool add blog_q += aq, then ACT exp directly on
  the logits (|b| <= ~0.8 so softmax needs no max shift).
phase2 (one q late, hides the softmax latency):
  DVE:   sm/rc/cc softmax tail; t2 = uhq * cc (ONE [128, 4096] op)
  PE:    psS += I2B^T @ t2 slices (s accumulation, 8x 213 ns)
psU bufs=3 and uhq bufs=6 keep PE/ACT running ahead through the
AllReduce windows.

Host-side layouts per core (W/u cast to bf16):
  WB  [32, 128, 1024]: WB[q, 16*p8+i, k*32+o] = W[q*8+p8, i, o*32+k]
  uB  [128, 2048]:     uB[16*p8+i, q*64+b] = u[b, q*8+p8, i]  (pass A)
  uZP [128, 4096]:     uZP[32r+16h+i, q*128+64h+b] = u[b, q*8+2r+h, i]
  I2B [128, 64]:       stacked 64x64 identities, bf16 (h/b merge)

Input DMAs are issued round-robin from the SP and ACT sequencers (uB
first so pass-A matmuls can chase the per-q W tiles; uZP/I2B last since
they are not needed until iteration 0), trimming the startup serial
segment before the first AllReduce.
"""

import numpy as np

B = 64
N_FULL = 2048
D_IN = 16
N_OUT = 32
D_OUT = 32
J = N_OUT * D_OUT  # 1024
N_CORES = 8
NL = N_FULL // N_CORES  # 256 local capsules
QB = NL // 8  # 32 q-blocks

_CACHE = {}


def _pack_inputs(u, W):
    """Shard along n and build per-core SBUF-friendly layouts (bf16)."""
    import ml_dtypes
    bf = ml_dtypes.bfloat16
    I2B = np.tile(np.eye(B, dtype=np.float32), (2, 1)).astype(bf)
    in_maps = []
    for c in range(N_CORES):
        ul = u[:, c * NL:(c + 1) * NL, :]          # [64, 256, 16]
        Wl = W[c * NL:(c + 1) * NL]                # [256, 16, 1024]
        # (k,o) layout: j' = k*32 + o
        Wko = np.ascontiguousarray(
            Wl.reshape(NL, D_IN, N_OUT, D_OUT).transpose(0, 1, 3, 2)
            .reshape(NL, D_IN, J))
        WB = np.ascontiguousarray(
            Wko.reshape(QB, 8, D_IN, J).reshape(QB, 128, J)).astype(bf)
        uB = np.ascontiguousarray(
            ul.reshape(B, QB, 8, D_IN).transpose(2, 3, 1, 0)
            .reshape(128, QB * B)).astype(bf)
        # uZP[32r+16h+i, q*128+64h'+b] = u[b, q*8+2r+h, i] iff h==h'
        un = ul.reshape(B, QB, 4, 2, D_IN)  # [b, q, r, h, i]
        Z = np.zeros((4, 2, D_IN, QB, 2, B), dtype=np.float32)
        for h in range(2):
            Z[:, h, :, :, h, :] = un[:, :, :, h, :].transpose(2, 3, 1, 0)
        uZP = Z.reshape(128, QB * 2 * B).astype(bf)
        in_maps.append({"WB": WB, "uB": uB, "uZP": uZP, "I2B": I2B})
    return in_maps


def _build_program():
    import concourse.bass as bass
    import concourse.tile as tile
    from concourse import bacc, mybir

    f32 = mybir.dt.float32
    bf16 = mybir.dt.bfloat16
    AF = mybir.ActivationFunctionType
    ALU = mybir.AluOpType
    AX = mybir.AxisListType

    nc = bacc.Bacc("TRN2", target_bir_lowering=False, debug=False,
                   num_devices=N_CORES)
    WB_d = nc.dram_tensor("WB", [QB, 128, J], bf16, kind="ExternalInput").ap()
    uB_d = nc.dram_tensor("uB", [128, QB * B], bf16, kind="ExternalInput").ap()
    uZP_d = nc.dram_tensor("uZP", [128, QB * 2 * B], bf16,
                           kind="ExternalInput").ap()
    I2B_d = nc.dram_tensor("I2B", [128, B], bf16, kind="ExternalInput").ap()
    v_d = nc.dram_tensor("v_out", [B, J], f32, kind="ExternalOutput").ap()

    with tile.TileContext(nc) as tc:
        with (
            tc.tile_pool(name="wpool", bufs=1) as wpool,
            tc.tile_pool(name="state", bufs=1) as state,
            tc.tile_pool(name="uhp", bufs=6) as uhp,
            tc.tile_pool(name="tmpp", bufs=1) as tmpp,
            tc.tile_pool(name="scratch", bufs=2) as scratch,
            tc.tile_pool(name="smalls", bufs=3) as smalls,
            tc.tile_pool(name="pU", bufs=3, space="PSUM") as pU,
            tc.tile_pool(name="pS", bufs=1, space="PSUM") as pS,
            tc.tile_pool(name="dram", bufs=2, space="DRAM") as dram,
        ):
            # --- load inputs: uB first (pass A gate), W chased by psA,
            # uZP/I2B last (not needed until iter 0).  Issue round-robin
            # over SP/ACT/DVE sequencers so issue time doesn't gate. ---
            issuers = [nc.sync, nc.scalar]
            uB_t = state.tile([128, QB * B], bf16, tag="uB")
            nc.sync.dma_start(uB_t[:], uB_d[:])
            w_tiles = []
            for q in range(QB):
                wt = wpool.tile([128, J], bf16, tag=f"w{q}", name=f"w{q}")
                issuers[q % 2].dma_start(wt[:], WB_d[q])
                w_tiles.append(wt)
            uZP_t = state.tile([128, QB * 2 * B], bf16, tag="uZP")
            nc.scalar.dma_start(uZP_t[:], uZP_d[:])
            I2B_t = state.tile([128, B], bf16, tag="I2B")
            nc.sync.dma_start(I2B_t[:], I2B_d[:])

            # logits blog[64h+b, (q*4+r)*32+o] for n = q*8+2r+h
            blog = state.tile([128, NL // 2 * N_OUT], bf16, tag="blog")
            nc.gpsimd.memset(blog[:], 0.0)
            v_t = state.tile([B, J], f32, tag="v")
            v_bf = state.tile([128, J], bf16, tag="v_bf")
            v_bf2 = state.tile([128, 2 * J], bf16, tag="v_bf2")

            def ar_squash(merged_ps, scale, final=False):
                """merged [64,J] psum -> AllReduce -> squash -> v_t, v_bf4."""
                s_loc = scratch.tile([B, J], bf16, tag="st", bufs=1)
                nc.scalar.mul(s_loc[:], merged_ps[:], scale)
                bin_ = dram.tile([B, J], bf16, tag="bounce_in")
                bout = dram.tile([B, J], bf16, tag="bounce_out")
                nc.sync.dma_start(bin_[:], s_loc[:])
                nc.gpsimd.collective_compute(
                    "AllReduce", ALU.add,
                    replica_groups=[list(range(N_CORES))],
                    ins=[bin_.opt()], outs=[bout.opt()],
                )
                s_g = scratch.tile([B, J], bf16, tag="st2", bufs=1)
                nc.sync.dma_start(s_g[:], bout[:])
                # squash: v = s * sqrt(n2)/(1+n2);  (k,o): norm over k.
                # bf16 sq + one k-fold keep the chain ops in DVE 2x mode.
                sq = scratch.tile([B, J], bf16, tag="st3", bufs=1)
                nc.vector.tensor_mul(sq[:], s_g[:], s_g[:])
                sqh = scratch.tile([B, J // 2], bf16, tag="st4", bufs=1)
                nc.vector.tensor_add(sqh[:], sq[:, 0:512], sq[:, 512:1024])
                sqh2 = scratch.tile([B, J // 4], bf16, tag="st5", bufs=1)
                nc.vector.tensor_add(sqh2[:], sqh[:, 0:256], sqh[:, 256:512])
                n2 = smalls.tile([B, N_OUT], f32, tag="n2")
                nc.vector.reduce_sum(
                    n2[:], sqh2[:].rearrange("p (k o) -> p o k", o=N_OUT),
                    axis=AX.X)
                # Sqrt issued FIRST among the ACT ops: its LoadActFuncSet
                # (sqrt lives in a different act table than exp, 1283ns)
                # has no data deps, so issued ahead of the add it runs
                # during the collective instead of on the post-collective
                # critical chain.
                rt = smalls.tile([B, N_OUT], f32, tag="rt")
                nc.scalar.activation(rt[:], n2[:], AF.Sqrt)
                # the +1 on DVE so it runs in parallel with ACT's sqrt
                n2p1 = smalls.tile([B, N_OUT], f32, tag="n2p1")
                nc.vector.tensor_scalar_add(n2p1[:], n2[:], 1.0)
                rcp = smalls.tile([B, N_OUT], f32, tag="rcp")
                nc.vector.reciprocal(rcp[:], n2p1[:])
                if final:
                    scl = smalls.tile([B, N_OUT], f32, tag="scl")
                    nc.vector.tensor_mul(scl[:], rt[:], rcp[:])
                    nc.vector.tensor_mul(
                        v_t[:].rearrange("p (k o) -> p k o", o=N_OUT),
                        s_g[:].rearrange("p (k o) -> p k o", o=N_OUT),
                        scl[:].unsqueeze(1).broadcast_to([B, D_OUT, N_OUT]))
                else:
                    # intermediate iterations: bf16 scale produced natively
                    # (no cast hop) keeps the v mult in DVE 2x mode
                    scl_bf = smalls.tile([B, N_OUT], bf16, tag="sclbf")
                    nc.vector.tensor_mul(scl_bf[:], rt[:], rcp[:])
                    nc.vector.tensor_mul(
                        v_bf[0:B, :].rearrange("p (k o) -> p k o", o=N_OUT),
                        s_g[:].rearrange("p (k o) -> p k o", o=N_OUT),
                        scl_bf[:].unsqueeze(1).broadcast_to(
                            [B, D_OUT, N_OUT]))
                    nc.vector.tensor_copy(v_bf[B:2 * B, :], v_bf[0:B, :])
                    # doubled copy for the r2/r3 pair multiply (4x mode)
                    nc.vector.tensor_copy(
                        v_bf2[:].rearrange("p (c j) -> p c j", c=2),
                        v_bf[:].unsqueeze(1).broadcast_to([128, 2, J]))

            # ---- pass A: s1 = (1/32) * sum_n uhat ----
            psA = pS.tile([B, J], f32, tag="psS", bufs=1)
            for q in range(QB):
                for jh in range(2):
                    nc.tensor.matmul(
                        psA[:, jh * 512:(jh + 1) * 512],
                        lhsT=uB_t[:, q * B:(q + 1) * B],
                        rhs=w_tiles[q][:, jh * 512:(jh + 1) * 512],
                        start=(q == 0), stop=(q == QB - 1))
            ar_squash(psA, 1.0 / N_OUT)

            # ---- passes B, C ----
            for it in range(2):
                psS = pS.tile([B, J], f32, tag="psS", bufs=1)
                uh_live = {}
                ee_live = {}

                def phase1(q):
                    # q=0 leans DVE: right after the AllReduce window the
                    # Pool chain can't start until v_bf lands, so the
                    # first q's extra Pool chunk only stalls DVE's tails
                    n_pool = 1 if q == 0 else (2 if q % 2 == 0 else 1)
                    # aq collects the q's 4 aa slices; one batched blog add
                    aq = smalls.tile([128, 4 * N_OUT], bf16, tag="aq")
                    # one uh tile for the whole q; evacs fill slices
                    uhq = uhp.tile([128, 4 * J], bf16, tag="uhq")

                    for r in range(4):
                        psU = pU.tile([128, J], f32, tag="psU", bufs=3)
                        for jh in range(2):
                            nc.tensor.matmul(
                                psU[:, jh * 512:(jh + 1) * 512],
                                lhsT=uZP_t[32 * r:32 * r + 32,
                                           q * 2 * B:(q + 1) * 2 * B],
                                rhs=w_tiles[q][32 * r:32 * r + 32,
                                               jh * 512:(jh + 1) * 512],
                                start=True, stop=True,
                                tile_position=(32 * r, 0))
                        nc.scalar.mul(uhq[:, r * J:(r + 1) * J], psU[:], 1.0)

                    th2all = tmpp.tile([128, J], bf16, tag="th2a",
                                        bufs=3)
                    # Pool-chunks first: start Pool's long chains early
                    for r in range(n_pool):
                        uh = uhq[:, r * J:(r + 1) * J]
                        tmp = tmpp.tile([128, J], bf16, tag="tmp", bufs=3)
                        nc.gpsimd.tensor_mul(tmp[:], uh, v_bf[:])
                        th = tmpp.tile([128, J // 2], bf16, tag="th", bufs=3)
                        nc.gpsimd.tensor_add(
                            th[:], tmp[:, 0:512], tmp[:, 512:1024])
                        nc.gpsimd.tensor_add(
                            th2all[:, r * 256:(r + 1) * 256],
                            th[:, 0:256], th[:, 256:512])
                    # DVE-chunks: r2/r3 always both DVE -> their tmp
                    # mults land in one tmpall tile and the th/th2 folds
                    # run pair-wide (saves two op overheads per q); an r1
                    # DVE chunk (odd q) keeps the classic 3-op chain.
                    if n_pool == 1:
                        uh = uhq[:, J:2 * J]
                        tmp = tmpp.tile([128, J], bf16, tag="tmp", bufs=3)
                        nc.vector.tensor_mul(tmp[:], uh, v_bf[:])
                        th = tmpp.tile([128, J // 2], bf16, tag="thd",
                                       bufs=3)
                        nc.vector.tensor_add(
                            th[:], tmp[:, 0:512], tmp[:, 512:1024])
                        nc.vector.tensor_add(
                            th2all[:, 256:512], th[:, 0:256],
                            th[:, 256:512])
                    tmpall = tmpp.tile([128, 2 * J], bf16, tag="tmpd",
                                       bufs=1)
                    nc.vector.tensor_mul(tmpall[:], uhq[:, 2 * J:4 * J],
                                         v_bf2[:])
                    thp = tmpp.tile([128, J], bf16, tag="thp", bufs=1)
                    iv = tmpall[:].rearrange("p (c h x) -> p c h x",
                                             c=2, h=2, x=512)
                    nc.vector.tensor_add(
                        thp[:].rearrange("p (c x) -> p c x", c=2),
                        iv[:, :, 0], iv[:, :, 1])
                    iv = thp[:].rearrange("p (c h x) -> p c h x",
                                          c=2, h=2, x=256)
                    nc.vector.tensor_add(
                        th2all[:, 512:1024].rearrange(
                            "p (c x) -> p c x", c=2),
                        iv[:, :, 0], iv[:, :, 1])
                    # batched tail: three r-blocked k-halving folds
                    # [128, (4,256)] -> [128, (4,32)] = aq, all DVE 2x
                    f3 = tmpp.tile([128, J // 2], bf16, tag="f3", bufs=2)
                    iv = th2all[:].rearrange("p (r h x) -> p r h x",
                                             r=4, h=2, x=128)
                    nc.vector.tensor_add(
                        f3[:].rearrange("p (r x) -> p r x", r=4),
                        iv[:, :, 0], iv[:, :, 1])
                    f4 = tmpp.tile([128, J // 4], bf16, tag="f4", bufs=2)
                    iv = f3[:].rearrange("p (r h x) -> p r h x",
                                         r=4, h=2, x=64)
                    nc.vector.tensor_add(
                        f4[:].rearrange("p (r x) -> p r x", r=4),
                        iv[:, :, 0], iv[:, :, 1])
                    iv = f4[:].rearrange("p (r h x) -> p r h x",
                                         r=4, h=2, x=32)
                    nc.vector.tensor_add(
                        aq[:].rearrange("p (r x) -> p r x", r=4),
                        iv[:, :, 0], iv[:, :, 1])
                    # single batched logits update for the whole q,
                    # on DVE: aq's writer is the DVE fold tail, so this
                    # avoids a DVE->Pool->ACT round-trip on the exp chain
                    bq = blog[:, q * 4 * N_OUT:(q + 1) * 4 * N_OUT]
                    nc.vector.tensor_add(bq, bq, aq[:])
                    # exp straight off the logits: |b| <= ~0.8 so the
                    # softmax max-shift is unnecessary (shift invariance)
                    ee = smalls.tile([128, 4 * N_OUT], f32, tag="ee")
                    nc.scalar.activation(ee[:], bq, AF.Exp)
                    ee_live[q] = ee
                    uh_live[q] = uhq

                ee_live = {}

                def a_chain(eng, uhq, aq, r0, nr):
                    """Batched agreement chain over nr r-blocks starting
                    at r0: tmp = uh*v, 5 k-halving folds -> aq slice."""
                    tmp = tmpp.tile([128, nr * J], bf16, tag=f"tmp{nr}",
                                    name=f"tmp{nr}", bufs=2 if nr == 2 else 1)
                    eng.tensor_mul(tmp[:], uhq[:, r0 * J:(r0 + nr) * J],
                                   v_bf4[:, 0:nr * J])
                    cur = tmp
                    width = J
                    for lvl in range(5):
                        width //= 2
                        if lvl == 4:
                            nxt_ap = aq[:, r0 * N_OUT:(r0 + nr) * N_OUT]
                        else:
                            nxt = tmpp.tile([128, nr * width], bf16,
                                            tag=f"fold{nr}_{lvl}",
                                            name=f"fold{nr}_{lvl}", bufs=2)
                            nxt_ap = nxt[:]
                        iv = cur[:].rearrange("p (r h x) -> p r h x",
                                              r=nr, h=2, x=width)
                        eng.tensor_add(
                            nxt_ap.rearrange("p (r x) -> p r x", r=nr),
                            iv[:, :, 0], iv[:, :, 1])
                        if lvl < 4:
                            cur = nxt

                def phase1(q):
                    eng = nc.gpsimd if q in POOL_QS else nc.vector
                    # one uh tile for the whole q; evacs fill slices
                    uhq = uhp.tile([128, 4 * J], bf16, tag="uhq")
                    aq = smalls.tile([128, 4 * N_OUT], bf16, tag="aq")
                    for r in range(4):
                        psU = pU.tile([128, J], f32, tag="psU", bufs=3)
                        for jh in range(2):
                            nc.tensor.matmul(
                                psU[:, jh * 512:(jh + 1) * 512],
                                lhsT=uZP_t[32 * r:32 * r + 32,
                                           q * 2 * B:(q + 1) * 2 * B],
                                rhs=w_tiles[q][32 * r:32 * r + 32,
                                               jh * 512:(jh + 1) * 512],
                                start=True, stop=True,
                                tile_position=(32 * r, 0))
                        nc.scalar.mul(uhq[:, r * J:(r + 1) * J], psU[:], 1.0)
                        # DVE-q: launch the half-chain as soon as its two
                        # evacs are in, overlapping the remaining evacs.
                        if eng is nc.vector and r == 1:
                            a_chain(eng, uhq, aq, 0, 2)
                    if eng is nc.vector:
                        a_chain(eng, uhq, aq, 2, 2)
                    else:
                        a_chain(eng, uhq, aq, 0, 4)
                    # single batched logits update
                    bq = blog[:, q * 4 * N_OUT:(q + 1) * 4 * N_OUT]
                    eng.tensor_add(bq, bq, aq[:])
                    # exp straight off the logits: |b| <= ~0.8 so the
                    # softmax max-shift is unnecessary (shift invariance)
                    ee = smalls.tile([128, 4 * N_OUT], f32, tag="ee")
                    nc.scalar.activation(ee[:], bq, AF.Exp)
                    ee_live[q] = ee
                    uh_live[q] = uhq

                def phase2(q, last):
                    # softmax tail (exp already issued in phase1), t2, merge
                    uhq = uh_live.pop(q)
                    ee = ee_live.pop(q)
                    sm = smalls.tile([128, 4], f32, tag="sm")
                    nc.vector.reduce_sum(
                        sm[:], ee[:].rearrange("p (r o) -> p r o", o=N_OUT),
                        axis=AX.X)
                    rc = smalls.tile([128, 4], f32, tag="rc")
                    nc.vector.reciprocal(rc[:], sm[:])
                    cc = smalls.tile([128, 4 * N_OUT], bf16, tag="cc")
                    nc.vector.tensor_tensor(
                        cc[:].rearrange("p (r o) -> p r o", o=N_OUT),
                        ee[:].rearrange("p (r o) -> p r o", o=N_OUT),
                        rc[:].unsqueeze(2).broadcast_to([128, 4, N_OUT]),
                        op=ALU.mult)
                    if not last:
                        # batched t2 = uh * c for all 4 r at once (bf16 2x)
                        t2 = tmpp.tile([128, 4 * J], bf16, tag="t2", bufs=2)
                        nc.vector.tensor_tensor(
                            t2[:].rearrange("p (r k o) -> p r k o",
                                            k=D_OUT, o=N_OUT),
                            uhq[:].rearrange("p (r k o) -> p r k o",
                                             k=D_OUT, o=N_OUT),
                            cc[:].rearrange("p (r o) -> p r o", o=N_OUT)
                            .unsqueeze(2).broadcast_to(
                                [128, 4, D_OUT, N_OUT]),
                            op=ALU.mult)
                        for sl in range(8):
                            nc.tensor.matmul(
                                psS[:, (sl % 2) * 512:(sl % 2 + 1) * 512],
                                lhsT=I2B_t[:],
                                rhs=t2[:, sl * 512:(sl + 1) * 512],
                                start=(q == 0 and sl < 2),
                                stop=False)
                    else:
                        # final q: r-sliced t2 with interleaved psS pairs
                        # so the s accumulation (gating the AllReduce)
                        # completes ~1.6us earlier
                        for r4 in range(4):
                            t2s = tmpp.tile([128, J], bf16, tag="t2s",
                                            name="t2s", bufs=2)
                            nc.vector.tensor_tensor(
                                t2s[:].rearrange("p (k o) -> p k o",
                                                 o=N_OUT),
                                uhq[:, r4 * J:(r4 + 1) * J]
                                .rearrange("p (k o) -> p k o", o=N_OUT),
                                cc[:, r4 * N_OUT:(r4 + 1) * N_OUT]
                                .unsqueeze(1).broadcast_to(
                                    [128, D_OUT, N_OUT]),
                                op=ALU.mult)
                            for jh in range(2):
                                nc.tensor.matmul(
                                    psS[:, jh * 512:(jh + 1) * 512],
                                    lhsT=I2B_t[:],
                                    rhs=t2s[:, jh * 512:(jh + 1) * 512],
                                    start=False,
                                    stop=(r4 == 3))

                for q in range(QB):
                    phase1(q)
                    if q >= 1:
                        phase2(q - 1, last=False)
                phase2(QB - 1, last=True)
                ar_squash(psS, 1.0, final=(it == 1))

            nc.sync.dma_start(v_d[:], v_t[:])

    nc.compile()
    return nc


def _get_program():
    if "nc" not in _CACHE:
        _CACHE["nc"] = _build_program()
    return _CACHE["nc"]


def kernel(u, W):
    from concourse.bass_utils import run_bass_kernel_spmd

    nc = _get_program()
    in_maps = _pack_inputs(np.asarray(u, np.float32), np.asarray(W, np.float32))
    res = run_bass_kernel_spmd(nc, in_maps, list(range(N_CORES)))
    v = res.results[0]["v_out"]
    # (k,o) layout -> [b, o, k]
    return np.ascontiguousarray(
        v.reshape(B, D_OUT, N_OUT).transpose(0, 2, 1))


# revision 52
# speedup vs baseline: 1.0326x; 1.0227x over previous
"""Trainium2 Bass kernel for capsule-network dynamic routing.

Problem: u [64, 2048, 16], W [2048, 16, 1024] ->
  uhat = einsum('bni,nij->bnj', u, W)  (viewed [B, N, 32, 32])
  3 routing iterations (softmax over out-caps, squash) -> v [64, 32, 32]

Sharding: n (input capsules) split across 8 cores, 256 per core.
W slice stays SBUF-resident (bf16); uhat is recomputed on the PE each
routing pass (never materialized to HBM).  The per-iteration s-reduction
([64, 1024] partial sums) is AllReduced across cores.

Layout: j is stored k-major (j' = k*32 + o, "(k,o)") so the c-weighting
(t2 = uh * c) broadcasts c over k with a packed last dim (DVE 2x mode).

Per-core n indexing: n = q*8 + 2r + h (q: 32 W blocks, r: 4 PE row
groups, h: psU partition half).  One chunk = (q, r): a single matmul
[K=32 zero-block-diag, M=128, N=1024] produces psU[64h+b, (k,o)] for
both h at once (tile_position=(32r, 0)).

Pipeline per chunk (engines overlap across chunks; per-chunk a-paths
are single-engine to avoid cross-engine ping-pong stalls):
  PE:    psU [128, 1024] = uZP-block^T @ WB-block          (2x 213 ns)
  ACT:   uh = psU -> bf16 into uhq slice                    (1038 ns)
  Pool-chunks (2 or 1 of the 4 per q, alternating):
    Pool: tmp = uh*v_bf; two k-halving folds -> th2 [128, 256]
    DVE:  aa = reduce_k(th2)                                (327 ns)
  DVE-chunks (the rest):
    DVE:  tmp = uh*v_bf (2x 594); th = k-fold (327); aa = reduce (594)
  per q: ONE batched Pool add blog_q += aq, then ACT exp directly on
  the logits (|b| <= ~0.8 so softmax needs no max shift).
phase2 (one q late, hides the softmax latency):
  DVE:   sm/rc/cc softmax tail; t2 = uhq * cc (ONE [128, 4096] op)
  PE:    psS += I2B^T @ t2 slices (s accumulation, 8x 213 ns)
psU bufs=3 and uhq bufs=6 keep PE/ACT running ahead through the
AllReduce windows.

Host-side layouts per core (W/u cast to bf16):
  WB  [32, 128, 1024]: WB[q, 16*p8+i, k*32+o] = W[q*8+p8, i, o*32+k]
  uB  [128, 2048]:     uB[16*p8+i, q*64+b] = u[b, q*8+p8, i]  (pass A)
  uZP [128, 4096]:     uZP[32r+16h+i, q*128+64h+b] = u[b, q*8+2r+h, i]
  I2B [128, 64]:       stacked 64x64 identities, bf16 (h/b merge)

Input DMAs are issued round-robin from the SP and ACT sequencers (uB
first so pass-A matmuls can chase the per-q W tiles; uZP/I2B last since
they are not needed until iteration 0), trimming the startup serial
segment before the first AllReduce.
"""

import numpy as np

B = 64
N_FULL = 2048
D_IN = 16
N_OUT = 32
D_OUT = 32
J = N_OUT * D_OUT  # 1024
N_CORES = 8
NL = N_FULL // N_CORES  # 256 local capsules
QB = NL // 8  # 32 q-blocks

_CACHE = {}


def _pack_inputs(u, W):
    """Shard along n and build per-core SBUF-friendly layouts (bf16)."""
    import ml_dtypes
    bf = ml_dtypes.bfloat16
    I2B = np.tile(np.eye(B, dtype=np.float32), (2, 1)).astype(bf)
    in_maps = []
    for c in range(N_CORES):
        ul = u[:, c * NL:(c + 1) * NL, :]          # [64, 256, 16]
        Wl = W[c * NL:(c + 1) * NL]                # [256, 16, 1024]
        # (k,o) layout: j' = k*32 + o
        Wko = np.ascontiguousarray(
            Wl.reshape(NL, D_IN, N_OUT, D_OUT).transpose(0, 1, 3, 2)
            .reshape(NL, D_IN, J))
        WB = np.ascontiguousarray(
            Wko.reshape(QB, 8, D_IN, J).reshape(QB, 128, J)).astype(bf)
        uB = np.ascontiguousarray(
            ul.reshape(B, QB, 8, D_IN).transpose(2, 3, 1, 0)
            .reshape(128, QB * B)).astype(bf)
        # uZP[32r+16h+i, q*128+64h'+b] = u[b, q*8+2r+h, i] iff h==h'
        un = ul.reshape(B, QB, 4, 2, D_IN)  # [b, q, r, h, i]
        Z = np.zeros((4, 2, D_IN, QB, 2, B), dtype=np.float32)
        for h in range(2):
            Z[:, h, :, :, h, :] = un[:, :, :, h, :].transpose(2, 3, 1, 0)
        uZP = Z.reshape(128, QB * 2 * B).astype(bf)
        in_maps.append({"WB": WB, "uB": uB, "uZP": uZP, "I2B": I2B})
    return in_maps


def _build_program():
    import concourse.bass as bass
    import concourse.tile as tile
    from concourse import bacc, mybir

    f32 = mybir.dt.float32
    bf16 = mybir.dt.bfloat16
    AF = mybir.ActivationFunctionType
    ALU = mybir.AluOpType
    AX = mybir.AxisListType

    nc = bacc.Bacc("TRN2", target_bir_lowering=False, debug=False,
                   num_devices=N_CORES)
    WB_d = nc.dram_tensor("WB", [QB, 128, J], bf16, kind="ExternalInput").ap()
    uB_d = nc.dram_tensor("uB", [128, QB * B], bf16, kind="ExternalInput").ap()
    uZP_d = nc.dram_tensor("uZP", [128, QB * 2 * B], bf16,
                           kind="ExternalInput").ap()
    I2B_d = nc.dram_tensor("I2B", [128, B], bf16, kind="ExternalInput").ap()
    v_d = nc.dram_tensor("v_out", [B, J], f32, kind="ExternalOutput").ap()

    with tile.TileContext(nc) as tc:
        with (
            tc.tile_pool(name="wpool", bufs=1) as wpool,
            tc.tile_pool(name="state", bufs=1) as state,
            tc.tile_pool(name="uhp", bufs=6) as uhp,
            tc.tile_pool(name="tmpp", bufs=1) as tmpp,
            tc.tile_pool(name="scratch", bufs=2) as scratch,
            tc.tile_pool(name="smalls", bufs=3) as smalls,
            tc.tile_pool(name="pU", bufs=3, space="PSUM") as pU,
            tc.tile_pool(name="pS", bufs=1, space="PSUM") as pS,
            tc.tile_pool(name="dram", bufs=2, space="DRAM") as dram,
        ):
            # --- load inputs: uB first (pass A gate), W chased by psA,
            # uZP/I2B last (not needed until iter 0).  Issue round-robin
            # over SP/ACT/DVE sequencers so issue time doesn't gate. ---
            issuers = [nc.sync, nc.scalar]
            uB_t = state.tile([128, QB * B], bf16, tag="uB")
            nc.sync.dma_start(uB_t[:], uB_d[:])
            w_tiles = []
            for q in range(QB):
                wt = wpool.tile([128, J], bf16, tag=f"w{q}", name=f"w{q}")
                issuers[q % 2].dma_start(wt[:], WB_d[q])
                w_tiles.append(wt)
            uZP_t = state.tile([128, QB * 2 * B], bf16, tag="uZP")
            nc.scalar.dma_start(uZP_t[:], uZP_d[:])
            I2B_t = state.tile([128, B], bf16, tag="I2B")
            nc.sync.dma_start(I2B_t[:], I2B_d[:])

            # logits blog[64h+b, (q*4+r)*32+o] for n = q*8+2r+h
            blog = state.tile([128, NL // 2 * N_OUT], bf16, tag="blog")
            nc.gpsimd.memset(blog[:], 0.0)
            v_t = state.tile([B, J], f32, tag="v")
            v_bf = state.tile([128, J], bf16, tag="v_bf")
            v_bf2 = state.tile([128, 2 * J], bf16, tag="v_bf2")

            def ar_squash(merged_ps, scale, final=False):
                """merged [64,J] psum -> AllReduce -> squash -> v_t, v_bf4."""
                s_loc = scratch.tile([B, J], bf16, tag="st", bufs=1)
                nc.scalar.mul(s_loc[:], merged_ps[:], scale)
                bin_ = dram.tile([B, J], bf16, tag="bounce_in")
                bout = dram.tile([B, J], bf16, tag="bounce_out")
                nc.sync.dma_start(bin_[:], s_loc[:])
                nc.gpsimd.collective_compute(
                    "AllReduce", ALU.add,
                    replica_groups=[list(range(N_CORES))],
                    ins=[bin_.opt()], outs=[bout.opt()],
                )
                s_g = scratch.tile([B, J], bf16, tag="st2", bufs=1)
                nc.sync.dma_start(s_g[:], bout[:])
                # squash: v = s * sqrt(n2)/(1+n2);  (k,o): norm over k.
                # bf16 sq + one k-fold keep the chain ops in DVE 2x mode.
                sq = scratch.tile([B, J], bf16, tag="st3", bufs=1)
                nc.vector.tensor_mul(sq[:], s_g[:], s_g[:])
                sqh = scratch.tile([B, J // 2], bf16, tag="st4", bufs=1)
                nc.vector.tensor_add(sqh[:], sq[:, 0:512], sq[:, 512:1024])
                sqh2 = scratch.tile([B, J // 4], bf16, tag="st5", bufs=1)
                nc.vector.tensor_add(sqh2[:], sqh[:, 0:256], sqh[:, 256:512])
                n2 = smalls.tile([B, N_OUT], f32, tag="n2")
                nc.vector.reduce_sum(
                    n2[:], sqh2[:].rearrange("p (k o) -> p o k", o=N_OUT),
                    axis=AX.X)
                # Sqrt issued FIRST among the ACT ops: its LoadActFuncSet
                # (sqrt lives in a different act table than exp, 1283ns)
                # has no data deps, so issued ahead of the add it runs
                # during the collective instead of on the post-collective
                # critical chain.
                rt = smalls.tile([B, N_OUT], f32, tag="rt")
                nc.scalar.activation(rt[:], n2[:], AF.Sqrt)
                # the +1 on DVE so it runs in parallel with ACT's sqrt
                n2p1 = smalls.tile([B, N_OUT], f32, tag="n2p1")
                nc.vector.tensor_scalar_add(n2p1[:], n2[:], 1.0)
                rcp = smalls.tile([B, N_OUT], f32, tag="rcp")
                nc.vector.reciprocal(rcp[:], n2p1[:])
                if final:
                    scl = smalls.tile([B, N_OUT], f32, tag="scl")
                    nc.vector.tensor_mul(scl[:], rt[:], rcp[:])
                    nc.vector.tensor_mul(
                        v_t[:].rearrange("p (k o) -> p k o", o=N_OUT),
                        s_g[:].rearrange("p (k o) -> p k o", o=N_OUT),
                        scl[:].unsqueeze(1).broadcast_to([B, D_OUT, N_OUT]))
                else:
                    # intermediate iterations: bf16 scale produced natively
                    # (no cast hop) keeps the v mult in DVE 2x mode
                    scl_bf = smalls.tile([B, N_OUT], bf16, tag="sclbf")
                    nc.vector.tensor_mul(scl_bf[:], rt[:], rcp[:])
                    nc.vector.tensor_mul(
                        v_bf[0:B, :].rearrange("p (k o) -> p k o", o=N_OUT),
                        s_g[:].rearrange("p (k o) -> p k o", o=N_OUT),
                        scl_bf[:].unsqueeze(1).broadcast_to(
                            [B, D_OUT, N_OUT]))
                    nc.vector.tensor_copy(v_bf[B:2 * B, :], v_bf[0:B, :])
                    # doubled copy for the r2/r3 pair multiply (4x mode)
                    nc.vector.tensor_copy(
                        v_bf2[:].rearrange("p (c j) -> p c j", c=2),
                        v_bf[:].unsqueeze(1).broadcast_to([128, 2, J]))

            # ---- pass A: s1 = (1/32) * sum_n uhat ----
            psA = pS.tile([B, J], f32, tag="psS", bufs=1)
            for q in range(QB):
                for jh in range(2):
                    nc.tensor.matmul(
                        psA[:, jh * 512:(jh + 1) * 512],
                        lhsT=uB_t[:, q * B:(q + 1) * B],
                        rhs=w_tiles[q][:, jh * 512:(jh + 1) * 512],
                        start=(q == 0), stop=(q == QB - 1))
            ar_squash(psA, 1.0 / N_OUT)

            # ---- passes B, C ----
            for it in range(2):
                psS = pS.tile([B, J], f32, tag="psS", bufs=1)
                uh_live = {}
                ee_live = {}

                def phase1(q):
                    # q=0 leans DVE: right after the AllReduce window the
                    # Pool chain can't start until v_bf lands, so the
                    # first q's extra Pool chunk only stalls DVE's tails
                    # first two q's after the AllReduce run fully on
                    # DVE: Pool's chains start cold post-window and would
                    # gate the batched fold tail during the ramp
                    n_pool = (0 if q in (0, 1) else
                              2 if q % 2 == 0 else 1)
                    # aq collects the q's 4 aa slices; one batched blog add
                    aq = smalls.tile([128, 4 * N_OUT], bf16, tag="aq")
                    # one uh tile for the whole q; evacs fill slices
                    uhq = uhp.tile([128, 4 * J], bf16, tag="uhq")

                    for r in range(4):
                        psU = pU.tile([128, J], f32, tag="psU", bufs=3)
                        for jh in range(2):
                            nc.tensor.matmul(
                                psU[:, jh * 512:(jh + 1) * 512],
                                lhsT=uZP_t[32 * r:32 * r + 32,
                                           q * 2 * B:(q + 1) * 2 * B],
                                rhs=w_tiles[q][32 * r:32 * r + 32,
                                               jh * 512:(jh + 1) * 512],
                                start=True, stop=True,
                                tile_position=(32 * r, 0))
                        nc.scalar.mul(uhq[:, r * J:(r + 1) * J], psU[:], 1.0)

                    th2all = tmpp.tile([128, J], bf16, tag="th2a",
                                        bufs=3)
                    # Pool-chunks first: start Pool's long chains early
                    for r in range(n_pool):
                        uh = uhq[:, r * J:(r + 1) * J]
                        tmp = tmpp.tile([128, J], bf16, tag="tmp", bufs=3)
                        nc.gpsimd.tensor_mul(tmp[:], uh, v_bf[:])
                        th = tmpp.tile([128, J // 2], bf16, tag="th", bufs=3)
                        nc.gpsimd.tensor_add(
                            th[:], tmp[:, 0:512], tmp[:, 512:1024])
                        nc.gpsimd.tensor_add(
                            th2all[:, r * 256:(r + 1) * 256],
                            th[:, 0:256], th[:, 256:512])
                    # DVE-chunks: r2/r3 always both DVE -> their tmp
                    # mults land in one tmpall tile and the th/th2 folds
                    # run pair-wide (saves two op overheads per q); an r1
                    # DVE chunk (odd q) keeps the classic 3-op chain.
                    for rs in range(n_pool, 2):
                        uh = uhq[:, rs * J:(rs + 1) * J]
                        tmp = tmpp.tile([128, J], bf16, tag="tmp", bufs=3)
                        nc.vector.tensor_mul(tmp[:], uh, v_bf[:])
                        th = tmpp.tile([128, J // 2], bf16, tag="thd",
                                       bufs=3)
                        nc.vector.tensor_add(
                            th[:], tmp[:, 0:512], tmp[:, 512:1024])
                        nc.vector.tensor_add(
                            th2all[:, rs * 256:(rs + 1) * 256],
                            th[:, 0:256], th[:, 256:512])
                    tmpall = tmpp.tile([128, 2 * J], bf16, tag="tmpd",
                                       bufs=1)
                    nc.vector.tensor_mul(tmpall[:], uhq[:, 2 * J:4 * J],
                                         v_bf2[:])
                    thp = tmpp.tile([128, J], bf16, tag="thp", bufs=1)
                    iv = tmpall[:].rearrange("p (c h x) -> p c h x",
                                             c=2, h=2, x=512)
                    nc.vector.tensor_add(
                        thp[:].rearrange("p (c x) -> p c x", c=2),
                        iv[:, :, 0], iv[:, :, 1])
                    iv = thp[:].rearrange("p (c h x) -> p c h x",
                                          c=2, h=2, x=256)
                    nc.vector.tensor_add(
                        th2all[:, 512:1024].rearrange(
                            "p (c x) -> p c x", c=2),
                        iv[:, :, 0], iv[:, :, 1])
                    # batched tail: three r-blocked k-halving folds
                    # [128, (4,256)] -> [128, (4,32)] = aq, all DVE 2x
                    f3 = tmpp.tile([128, J // 2], bf16, tag="f3", bufs=2)
                    iv = th2all[:].rearrange("p (r h x) -> p r h x",
                                             r=4, h=2, x=128)
                    nc.vector.tensor_add(
                        f3[:].rearrange("p (r x) -> p r x", r=4),
                        iv[:, :, 0], iv[:, :, 1])
                    f4 = tmpp.tile([128, J // 4], bf16, tag="f4", bufs=2)
                    iv = f3[:].rearrange("p (r h x) -> p r h x",
                                         r=4, h=2, x=64)
                    nc.vector.tensor_add(
                        f4[:].rearrange("p (r x) -> p r x", r=4),
                        iv[:, :, 0], iv[:, :, 1])
                    iv = f4[:].rearrange("p (r h x) -> p r h x",
                                         r=4, h=2, x=32)
                    nc.vector.tensor_add(
                        aq[:].rearrange("p (r x) -> p r x", r=4),
                        iv[:, :, 0], iv[:, :, 1])
                    # single batched logits update for the whole q,
                    # on DVE: aq's writer is the DVE fold tail, so this
                    # avoids a DVE->Pool->ACT round-trip on the exp chain
                    bq = blog[:, q * 4 * N_OUT:(q + 1) * 4 * N_OUT]
                    nc.vector.tensor_add(bq, bq, aq[:])
                    # exp straight off the logits: |b| <= ~0.8 so the
                    # softmax max-shift is unnecessary (shift invariance)
                    ee = smalls.tile([128, 4 * N_OUT], f32, tag="ee")
                    nc.scalar.activation(ee[:], bq, AF.Exp)
                    ee_live[q] = ee
                    uh_live[q] = uhq

                ee_live = {}

                def a_chain(eng, uhq, aq, r0, nr):
                    """Batched agreement chain over nr r-blocks starting
                    at r0: tmp = uh*v, 5 k-halving folds -> aq slice."""
                    tmp = tmpp.tile([128, nr * J], bf16, tag=f"tmp{nr}",
                                    name=f"tmp{nr}", bufs=2 if nr == 2 else 1)
                    eng.tensor_mul(tmp[:], uhq[:, r0 * J:(r0 + nr) * J],
                                   v_bf4[:, 0:nr * J])
                    cur = tmp
                    width = J
                    for lvl in range(5):
                        width //= 2
                        if lvl == 4:
                            nxt_ap = aq[:, r0 * N_OUT:(r0 + nr) * N_OUT]
                        else:
                            nxt = tmpp.tile([128, nr * width], bf16,
                                            tag=f"fold{nr}_{lvl}",
                                            name=f"fold{nr}_{lvl}", bufs=2)
                            nxt_ap = nxt[:]
                        iv = cur[:].rearrange("p (r h x) -> p r h x",
                                              r=nr, h=2, x=width)
                        eng.tensor_add(
                            nxt_ap.rearrange("p (r x) -> p r x", r=nr),
                            iv[:, :, 0], iv[:, :, 1])
                        if lvl < 4:
                            cur = nxt

                def phase1(q):
                    eng = nc.gpsimd if q in POOL_QS else nc.vector
                    # one uh tile for the whole q; evacs fill slices
                    uhq = uhp.tile([128, 4 * J], bf16, tag="uhq")
                    aq = smalls.tile([128, 4 * N_OUT], bf16, tag="aq")
                    for r in range(4):
                        psU = pU.tile([128, J], f32, tag="psU", bufs=3)
                        for jh in range(2):
                            nc.tensor.matmul(
                                psU[:, jh * 512:(jh + 1) * 512],
                                lhsT=uZP_t[32 * r:32 * r + 32,
                                           q * 2 * B:(q + 1) * 2 * B],
                                rhs=w_tiles[q][32 * r:32 * r + 32,
                                               jh * 512:(jh + 1) * 512],
                                start=True, stop=True,
                                tile_position=(32 * r, 0))
                        nc.scalar.mul(uhq[:, r * J:(r + 1) * J], psU[:], 1.0)
                        # DVE-q: launch the half-chain as soon as its two
                        # evacs are in, overlapping the remaining evacs.
                        if eng is nc.vector and r == 1:
                            a_chain(eng, uhq, aq, 0, 2)
                    if eng is nc.vector:
                        a_chain(eng, uhq, aq, 2, 2)
                    else:
                        a_chain(eng, uhq, aq, 0, 4)
                    # single batched logits update
                    bq = blog[:, q * 4 * N_OUT:(q + 1) * 4 * N_OUT]
                    eng.tensor_add(bq, bq, aq[:])
                    # exp straight off the logits: |b| <= ~0.8 so the
                    # softmax max-shift is unnecessary (shift invariance)
                    ee = smalls.tile([128, 4 * N_OUT], f32, tag="ee")
                    nc.scalar.activation(ee[:], bq, AF.Exp)
                    ee_live[q] = ee
                    uh_live[q] = uhq

                def phase2(q, last):
                    # softmax tail (exp already issued in phase1), t2, merge
                    uhq = uh_live.pop(q)
                    ee = ee_live.pop(q)
                    sm = smalls.tile([128, 4], f32, tag="sm")
                    nc.vector.reduce_sum(
                        sm[:], ee[:].rearrange("p (r o) -> p r o", o=N_OUT),
                        axis=AX.X)
                    rc = smalls.tile([128, 4], f32, tag="rc")
                    nc.vector.reciprocal(rc[:], sm[:])
                    cc = smalls.tile([128, 4 * N_OUT], bf16, tag="cc")
                    nc.vector.tensor_tensor(
                        cc[:].rearrange("p (r o) -> p r o", o=N_OUT),
                        ee[:].rearrange("p (r o) -> p r o", o=N_OUT),
                        rc[:].unsqueeze(2).broadcast_to([128, 4, N_OUT]),
                        op=ALU.mult)
                    if not last:
                        # batched t2 = uh * c for all 4 r at once (bf16 2x)
                        t2 = tmpp.tile([128, 4 * J], bf16, tag="t2", bufs=2)
                        nc.vector.tensor_tensor(
                            t2[:].rearrange("p (r k o) -> p r k o",
                                            k=D_OUT, o=N_OUT),
                            uhq[:].rearrange("p (r k o) -> p r k o",
                                             k=D_OUT, o=N_OUT),
                            cc[:].rearrange("p (r o) -> p r o", o=N_OUT)
                            .unsqueeze(2).broadcast_to(
                                [128, 4, D_OUT, N_OUT]),
                            op=ALU.mult)
                        for sl in range(8):
                            nc.tensor.matmul(
                                psS[:, (sl % 2) * 512:(sl % 2 + 1) * 512],
                                lhsT=I2B_t[:],
                                rhs=t2[:, sl * 512:(sl + 1) * 512],
                                start=(q == 0 and sl < 2),
                                stop=False)
                    else:
                        # final q: r-sliced t2 with interleaved psS pairs
                        # so the s accumulation (gating the AllReduce)
                        # completes ~1.6us earlier
                        for r4 in range(4):
                            t2s = tmpp.tile([128, J], bf16, tag="t2s",
                                            name="t2s", bufs=2)
                            nc.vector.tensor_tensor(
                                t2s[:].rearrange("p (k o) -> p k o",
                                                 o=N_OUT),
                                uhq[:, r4 * J:(r4 + 1) * J]
                                .rearrange("p (k o) -> p k o", o=N_OUT),
                                cc[:, r4 * N_OUT:(r4 + 1) * N_OUT]
                                .unsqueeze(1).broadcast_to(
                                    [128, D_OUT, N_OUT]),
                                op=ALU.mult)
                            for jh in range(2):
                                nc.tensor.matmul(
                                    psS[:, jh * 512:(jh + 1) * 512],
                                    lhsT=I2B_t[:],
                                    rhs=t2s[:, jh * 512:(jh + 1) * 512],
                                    start=False,
                                    stop=(r4 == 3))

                for q in range(QB):
                    phase1(q)
                    if q >= 1:
                        phase2(q - 1, last=False)
                phase2(QB - 1, last=True)
                ar_squash(psS, 1.0, final=(it == 1))

            nc.sync.dma_start(v_d[:], v_t[:])

    nc.compile()
    return nc


def _get_program():
    if "nc" not in _CACHE:
        _CACHE["nc"] = _build_program()
    return _CACHE["nc"]


def kernel(u, W):
    from concourse.bass_utils import run_bass_kernel_spmd

    nc = _get_program()
    in_maps = _pack_inputs(np.asarray(u, np.float32), np.asarray(W, np.float32))
    res = run_bass_kernel_spmd(nc, in_maps, list(range(N_CORES)))
    v = res.results[0]["v_out"]
    # (k,o) layout -> [b, o, k]
    return np.ascontiguousarray(
        v.reshape(B, D_OUT, N_OUT).transpose(0, 2, 1))
